# revision 1
# baseline (speedup 1.0000x reference)
"""Self-contained Trainium2 (Bass/Tile) multi-head-attention kernel.

Problem: nn_Attention — B=2, N=2048, E=1024, H=16 heads, D=64, fp32 I/O.

    out = softmax((q@Wq.T+bq) (k@Wk.T+bk)^T / sqrt(D)) (v@Wv.T+bv) @ Wo.T + bo

Distribution over 8 NeuronCores: data-parallel over batch (2 groups of 4
cores) x tensor-parallel over heads (4 heads / 256 features per core) — no
collectives. Each core computes its heads' full contribution to the output
projection (a [N, E] partial sum); the host sums the four partials per
batch and adds the bias terms (bo plus the folded v-bias Wo@bv) in fp32.

Per-core algorithm (all activations kept in a transposed [features, tokens]
layout so no on-chip transposes are ever needed):
  - q/k projections emit qpT/kpT [256, N] directly (weights stationary).
  - scores_T[j, i] = khT.T @ qhT per head; exp on ScalarE (1/sqrt(D) folded
    into the activation's pre-scale).
  - AV: U_T = [vp | 1]^T @ expS_T yields the attention output AND the
    softmax row sums in one accumulation (ones column trick).
  - normalization: fast approx reciprocal of the sums row, broadcast across
    partitions, multiply.
  - out[i, e] = att_T.T @ WoT accumulated over the 256 local features.
Matmul operands are bf16 (cast on host; fp32 PSUM accumulation everywhere);
end-to-end rel err vs the fp32 reference is ~7e-3.

Performance structure: score-PSUM double-buffered across two asymmetric
bank groups (4+2 banks, jt-groups of 4,2,4,2,4), previous iteration's AV
matmuls and the previous i-block's out-projection tiles interleave as
TensorEngine filler during exp(), keeping the PE clock (HAM) warm.
"""

import numpy as np

from collections import deque

import concourse.mybir as mybir
import concourse.tile as tile
from concourse import bacc

F32_NP = np.float32
B, H = 2, 16

F32 = mybir.dt.float32
BF16 = mybir.dt.bfloat16

P = 128
N = 2048
E = 1024
FL = 256
HLOC = 4
D = 64
ECH = E // P      # 8
NB = 512
NBLK = N // NB    # 4
NT = N // P       # 16
JT = N // P       # 16
SCALE = 0.125

# (tag, first jt, n jts) — score-group schedule within one (ib, h)
SC_GROUPS = [("pssA", 0, 4), ("pssB", 4, 2), ("pssA", 6, 4),
             ("pssB", 10, 2), ("pssA", 12, 4)]


def build():
    nc = bacc.Bacc("TRN2", target_bir_lowering=False, debug=False,
                   enable_asserts=True, num_devices=8)

    d_qT = nc.dram_tensor("qT", [E, N], BF16, kind="ExternalInput")
    d_kT = nc.dram_tensor("kT", [E, N], BF16, kind="ExternalInput")
    d_vT = nc.dram_tensor("vT", [E, N], BF16, kind="ExternalInput")
    d_wq = nc.dram_tensor("wq", [E, FL], BF16, kind="ExternalInput")
    d_wk = nc.dram_tensor("wk", [E, FL], BF16, kind="ExternalInput")
    d_wv = nc.dram_tensor("wv", [E, FL], BF16, kind="ExternalInput")
    d_wo = nc.dram_tensor("wo", [FL, E], BF16, kind="ExternalInput")
    d_bq = nc.dram_tensor("bq", [P, 2], F32, kind="ExternalInput")
    d_bk = nc.dram_tensor("bk", [P, 2], F32, kind="ExternalInput")
    d_out = nc.dram_tensor("out", [N, E], F32, kind="ExternalOutput")

    with tile.TileContext(nc) as tc:
        _body(nc, tc, d_qT, d_kT, d_vT, d_wq, d_wk, d_wv, d_wo,
              d_bq, d_bk, d_out)

    nc.compile()
    return nc


def _body(nc, tc, d_qT, d_kT, d_vT, d_wq, d_wk, d_wv, d_wo,
          d_bq, d_bk, d_out):
    exp_f = mybir.ActivationFunctionType.Exp
    mult = mybir.AluOpType.mult

    with (
        tc.tile_pool(name="weights", bufs=1) as wpool,
        tc.tile_pool(name="acts", bufs=1) as apool,
        tc.tile_pool(name="stream", bufs=6) as spool,
        tc.tile_pool(name="exps", bufs=3) as epool,
        tc.tile_pool(name="small", bufs=4) as mpool,
    ):
        # ---- resident weights (already bf16 from host) ----
        wq_sb = wpool.tile([P, ECH, FL], BF16, tag="wq_sb")
        wk_sb = wpool.tile([P, ECH, FL], BF16, tag="wk_sb")
        wv_sb = wpool.tile([P, ECH, FL], BF16, tag="wv_sb")
        wo_sb = wpool.tile([P, 2, E], BF16, tag="wo_sb")
        nc.sync.dma_start(wq_sb[:], d_wq.ap().rearrange("(c p) f -> p c f", p=P))
        nc.sync.dma_start(wk_sb[:], d_wk.ap().rearrange("(c p) f -> p c f", p=P))
        nc.sync.dma_start(wv_sb[:], d_wv.ap().rearrange("(c p) f -> p c f", p=P))
        nc.sync.dma_start(wo_sb[:], d_wo.ap().rearrange("(t p) e -> p t e", p=P))
        bq_sb = wpool.tile([P, 2], F32, tag="bq_sb")
        bk_sb = wpool.tile([P, 2], F32, tag="bk_sb")
        nc.sync.dma_start(bq_sb[:], d_bq.ap())
        nc.sync.dma_start(bk_sb[:], d_bk.ap())

        # ---- persistent activations ----
        qp_sb = apool.tile([P, 2, N], BF16, tag="qp_sb")
        kp_sb = apool.tile([P, 2, N], BF16, tag="kp_sb")
        vp_sb = apool.tile([P, NT, HLOC * (D + 1)], BF16, tag="vp_sb")
        att_sb = apool.tile([P, 2, N], BF16, tag="att_sb")

        # ---- projections: k, v, q (nb-outer; LDW overlaps streaming) ----
        with tc.tile_pool(name="ps_proj", bufs=4, space="PSUM") as pproj, \
             tc.tile_pool(name="scratch", bufs=1, space="DRAM") as dpool:
            # PE warm-up: ~4us of dummy matmuls opens the HAM clock gate
            # before the first real projection matmul; the result is DMA'd
            # to a scratch DRAM tile so nothing elides the chain.
            warm_sb = wpool.tile([P, NB], BF16, tag="warm_sb")
            nc.vector.memset(warm_sb[:], 0.0)
            warm_ps = pproj.tile([P, NB], F32, tag="pq", name="warm_ps")
            for i in range(18):
                nc.tensor.matmul(warm_ps[:], warm_sb[:, 0:P], warm_sb[:],
                                 start=(i == 0), stop=(i == 17))
            wdump = mpool.tile([1, NB], F32, tag="wdump", name="wdump")
            nc.vector.tensor_copy(wdump[:], warm_ps[0:1, :])
            wdram = dpool.tile([1, NB], F32, tag="wdram", name="wdram")
            nc.sync.dma_start(wdram[:], wdump[:])
            for phase, src, wsb, bias, dst in (
                ("proj_k", d_kT, wk_sb, bk_sb, kp_sb),
                ("proj_q", d_qT, wq_sb, bq_sb, qp_sb),
            ):
                with nc.named_scope(phase):
                    for nb in range(NBLK):
                        pst = [pproj.tile([P, NB], F32, tag="pq",
                                          name=f"pq{ft}") for ft in range(2)]
                        xt = spool.tile([P, ECH, NB], BF16, tag="xt", name="xt")
                        for ec in range(ECH):
                            # spread input DMAs across three DMA paths:
                            # both HWDGE queues (SP + idle Activation) plus
                            # the GpSimd SWDGE channel
                            eng = (nc.sync, nc.scalar, nc.gpsimd)[ec % 3]
                            eng.dma_start(
                                xt[:, ec, :],
                                src.ap()[ec * P:(ec + 1) * P,
                                         nb * NB:(nb + 1) * NB])
                        for ec in range(ECH):
                            for ft in range(2):
                                nc.tensor.matmul(
                                    pst[ft][:],
                                    wsb[:, ec, ft * P:(ft + 1) * P],
                                    xt[:, ec, :],
                                    start=(ec == 0), stop=(ec == ECH - 1))
                        for ft in range(2):
                            nc.vector.tensor_scalar_add(
                                dst[:, ft, nb * NB:(nb + 1) * NB],
                                pst[ft][:], bias[:, ft:ft + 1])
                if phase == "proj_k":
                    with nc.named_scope("proj_v"):
                        for nb in range(NBLK):
                            vt3 = spool.tile([P, ECH, NB], BF16, tag="xt",
                                             name="vt3")
                            for ec in range(ECH):
                                eng = (nc.sync, nc.scalar, nc.gpsimd)[ec % 3]
                                eng.dma_start(
                                    vt3[:, ec, :],
                                    d_vT.ap()[ec * P:(ec + 1) * P,
                                              nb * NB:(nb + 1) * NB])
                            for sub in range(NB // P):
                                nt_i = nb * (NB // P) + sub
                                psv = pproj.tile([P, FL], F32, tag="pq",
                                                 padded_shape=[P, NB],
                                                 name="psv")
                                for ec in range(ECH):
                                    nc.tensor.matmul(
                                        psv[:],
                                        vt3[:, ec, sub * P:(sub + 1) * P],
                                        wv_sb[:, ec, :],
                                        start=(ec == 0), stop=(ec == ECH - 1))
                                vslc = vp_sb[:, nt_i]
                                nc.vector.tensor_copy(
                                    vslc.rearrange(
                                        "p (h x) -> p h x", h=HLOC)[:, :, 0:D],
                                    psv[:].rearrange("p (h x) -> p h x",
                                                     h=HLOC))
                                nc.vector.memset(
                                    vslc.rearrange(
                                        "p (h x) -> p h x", h=HLOC)[:, :,
                                                                    D:D + 1],
                                    1.0)

        # ---- attention + out-proj, software-pipelined ----
        with tc.tile_pool(name="ps_attn", bufs=1, space="PSUM") as pattn:
            iters = [(ib, h) for ib in range(NBLK) for h in range(HLOC)]
            state = {}
            oproj_q = deque()

            def sc_group(idx, gi):
                ib, h = iters[idx]
                st = state[idx]
                tag, jt0, njt = SC_GROUPS[gi]
                pofs = (h % 2) * D
                ft = h // 2
                qh = qp_sb[pofs:pofs + D, ft, ib * NB:(ib + 1) * NB]
                pss = pattn.tile([P, njt * NB], F32, tag=tag, name="pss")
                for u in range(njt):
                    jt = jt0 + u
                    kh = kp_sb[pofs:pofs + D, ft, jt * P:(jt + 1) * P]
                    nc.tensor.matmul(pss[:, u * NB:(u + 1) * NB], kh, qh,
                                     start=True, stop=True)
                nc.scalar.activation(
                    st["exps"][:, jt0:jt0 + njt, :]
                        .rearrange("p a b -> p (a b)"),
                    pss[:], exp_f, scale=SCALE)

            def av_group(idx, g):
                ib, h = iters[idx]
                st = state[idx]
                if "pu" not in st:
                    st["pu"] = pattn.tile([D + 1, NB], F32, tag="pu", bufs=1,
                                          padded_shape=[P, NB], name="pu")
                for u in range(4):
                    jt = g * 4 + u
                    nc.tensor.matmul(
                        st["pu"][:],
                        vp_sb[:, jt, h * (D + 1):(h + 1) * (D + 1)],
                        st["exps"][:, jt, :],
                        start=(jt == 0), stop=(jt == JT - 1))

            def norm(idx):
                ib, h = iters[idx]
                st = state[idx]
                pofs = (h % 2) * D
                ft = h // 2
                pu = st["pu"]
                u_sb = mpool.tile([D + 1, NB], F32, tag="u_sb", name="u_sb")
                nc.vector.tensor_copy(u_sb[:], pu[:])
                srow = mpool.tile([1, NB], F32, tag="srow", name="srow")
                nc.vector.tensor_copy(srow[:], u_sb[D:D + 1, :])
                rec = mpool.tile([1, NB], F32, tag="rec", name="rec")
                nc.vector.reciprocal_approx_fast(rec[:], srow[:])
                rb = mpool.tile([D, NB], F32, tag="rb", name="rb")
                nc.gpsimd.partition_broadcast(rb[:], rec[:])
                nc.vector.tensor_tensor(
                    att_sb[pofs:pofs + D, ft, ib * NB:(ib + 1) * NB],
                    u_sb[0:D, :], rb[:], op=mult)
                if h == HLOC - 1:
                    oproj_q.extend((ib * (NB // P) + s, eb)
                                   for s in range(NB // P) for eb in range(2))

            def oproj_half():
                if not oproj_q:
                    return
                it, eb = oproj_q.popleft()
                po = pattn.tile([P, NB], F32, tag="po", bufs=1, name="po")
                for ft2 in range(2):
                    nc.tensor.matmul(
                        po[:],
                        att_sb[:, ft2, it * P:(it + 1) * P],
                        wo_sb[:, ft2, eb * NB:(eb + 1) * NB],
                        start=(ft2 == 0), stop=(ft2 == 1))
                ot = mpool.tile([P, NB], F32, tag="ot", name="ot")
                nc.vector.tensor_copy(ot[:], po[:])
                nc.sync.dma_start(
                    d_out.ap()[it * P:(it + 1) * P,
                               eb * NB:(eb + 1) * NB],
                    ot[:])

            for idx in range(len(iters)):
                with nc.named_scope(f"attn_i{idx}"):
                    state[idx] = {
                        "exps": epool.tile([P, JT, NB], BF16, tag="exps",
                                           name="exps"),
                    }
                    sc_group(idx, 0)
                    if idx > 0:
                        av_group(idx - 1, 2)
                    sc_group(idx, 1)
                    if idx > 0:
                        av_group(idx - 1, 3)
                        norm(idx - 1)
                    sc_group(idx, 2)
                    av_group(idx, 0)
                    oproj_half()
                    sc_group(idx, 3)
                    av_group(idx, 1)
                    oproj_half()
                    sc_group(idx, 4)
                    if idx >= 2:
                        del state[idx - 2]
            last = len(iters) - 1
            av_group(last, 2)
            av_group(last, 3)
            norm(last)
            while oproj_q:
                oproj_half()


_CACHE = {}


def _shard_inputs(q, k, v, Wq, bq, Wk, bk, Wv, Wo):
    import ml_dtypes
    bf = ml_dtypes.bfloat16
    in_maps = []
    for c in range(8):
        b, g = divmod(c, 4)
        fs = slice(g * FL, (g + 1) * FL)
        in_maps.append({
            "qT": np.ascontiguousarray(q[b].T.astype(bf)),
            "kT": np.ascontiguousarray(k[b].T.astype(bf)),
            "vT": np.ascontiguousarray(v[b].T.astype(bf)),
            "wq": np.ascontiguousarray(Wq[fs, :].T.astype(bf)),
            "wk": np.ascontiguousarray(Wk[fs, :].T.astype(bf)),
            "wv": np.ascontiguousarray(Wv[fs, :].T.astype(bf)),
            "wo": np.ascontiguousarray(Wo[:, fs].T.astype(bf)),
            "bq": np.ascontiguousarray(bq[fs].reshape(2, P).T.astype(F32_NP)),
            "bk": np.ascontiguousarray(bk[fs].reshape(2, P).T.astype(F32_NP)),
        })
    return in_maps


def kernel(q, k, v, Wq, bq, Wk, bk, Wv, bv, Wo, bo):
    from concourse import bass_utils

    q = np.asarray(q, F32_NP)
    k = np.asarray(k, F32_NP)
    v = np.asarray(v, F32_NP)
    Wq = np.asarray(Wq, F32_NP)
    Wk = np.asarray(Wk, F32_NP)
    Wv = np.asarray(Wv, F32_NP)
    Wo = np.asarray(Wo, F32_NP)
    bq = np.asarray(bq, F32_NP)
    bk = np.asarray(bk, F32_NP)
    bv = np.asarray(bv, F32_NP)
    bo = np.asarray(bo, F32_NP)

    if "nc" not in _CACHE:
        _CACHE["nc"] = build()
    nc = _CACHE["nc"]

    in_maps = _shard_inputs(q, k, v, Wq, bq, Wk, bk, Wv, Wo)
    res = bass_utils.run_bass_kernel_spmd(nc, in_maps, core_ids=list(range(8)))

    extra = (Wo @ bv + bo).astype(F32_NP)
    out = np.zeros((B, N, E), F32_NP)
    for b in range(B):
        out[b] = (res.results[b * 4 + 0]["out"] + res.results[b * 4 + 1]["out"]
                  + res.results[b * 4 + 2]["out"] + res.results[b * 4 + 3]["out"]
                  + extra)
    return out



# revision 4
# speedup vs baseline: 1.3707x; 1.3707x over previous
"""Self-contained Trainium2 (Bass/Tile) multi-head-attention kernel.

Problem: nn_Attention — B=2, N=2048, E=1024, H=16 heads, D=64, fp32 I/O.

    out = softmax((q@Wq.T+bq) (k@Wk.T+bk)^T / sqrt(D)) (v@Wv.T+bv) @ Wo.T + bo

Distribution over 8 NeuronCores: data-parallel over batch (2 groups of 4
cores) x tensor-parallel over heads (4 heads / 256 features per core) — no
collectives. Each core computes its heads' full contribution to the output
projection (a [N, E] partial sum, written bf16); the host sums the four
partials per batch in fp32 and adds the bias terms (bo plus the folded
v-bias Wo@bv).

Per-core algorithm (activations kept in a transposed [features, tokens]
layout so no on-chip transposes are needed):
  - k/v projections stream in, then q block 0; q blocks 1-3 are projected
    *inside* the attention loop (their PSUM bank is the out-proj bank,
    which is idle during the first two attention iterations).
  - attention runs over 8 pair-iterations (4 query blocks x 2 head pairs).
    The two heads of a pair live at SBUF partitions 0-63 / 64-127, so their
    score matmuls carry PE tile_position (0,0) and (64,0) and execute
    CONCURRENTLY on the two halves of the systolic array (the contraction
    dim is only D=64). Scores land in 4-bank / 2-bank PSUM groups holding
    both heads; one exp() ACTIVATE per group keeps ScalarE — the actual
    bottleneck at ~16.3us/pair-iter — at maximum instruction size.
  - AV (ones-column trick folds the softmax row sums into the same
    accumulation) and the normalization lag one iteration behind, acting as
    TensorE filler under the exp()s with no ACT-dependency stalls; a single
    shared PSUM bank serves both heads' AV sequentially.
  - out-projection halves drain as filler once a block's four heads are
    normalized; the tail rotates through three freed PSUM banks.
Matmul operands are bf16 (cast on host; fp32 PSUM accumulation everywhere).
"""

import numpy as np

from collections import deque

import concourse.mybir as mybir
import concourse.tile as tile
from concourse import bacc

F32_NP = np.float32
B, H = 2, 16

F32 = mybir.dt.float32
BF16 = mybir.dt.bfloat16

P = 128
N = 2048
E = 1024
FL = 256
HLOC = 4
D = 64
ECH = E // P      # 8
NB = 512
NBLK = N // NB    # 4
NT = N // P       # 16
JT = N // P       # 16
SCALE = 0.125

# (psum tag, first jt, n jts) — score-group schedule within one pair-iter.
# "psA" is a 4-bank group (2 jt x 2 heads), "psB" a 2-bank group (1 jt x 2).
SC_GROUPS = [("psA", 0, 2), ("psB", 2, 1), ("psA", 3, 2), ("psB", 5, 1),
             ("psA", 6, 2), ("psB", 8, 1), ("psA", 9, 2), ("psB", 11, 1),
             ("psA", 12, 2), ("psB", 14, 1), ("psA", 15, 1)]

ITERS = [(ib, f) for ib in range(NBLK) for f in range(2)]  # idx = 2*ib + f


def build():
    nc = bacc.Bacc("TRN2", target_bir_lowering=False, debug=False,
                   enable_asserts=True, num_devices=8)

    d_qT = nc.dram_tensor("qT", [E, N], BF16, kind="ExternalInput")
    d_kT = nc.dram_tensor("kT", [E, N], BF16, kind="ExternalInput")
    d_vT = nc.dram_tensor("vT", [E, N], BF16, kind="ExternalInput")
    d_wq = nc.dram_tensor("wq", [E, FL], BF16, kind="ExternalInput")
    d_wk = nc.dram_tensor("wk", [E, FL], BF16, kind="ExternalInput")
    d_wv = nc.dram_tensor("wv", [E, FL], BF16, kind="ExternalInput")
    d_wo = nc.dram_tensor("wo", [FL, E], BF16, kind="ExternalInput")
    d_bq = nc.dram_tensor("bq", [P, 2], F32, kind="ExternalInput")
    d_bk = nc.dram_tensor("bk", [P, 2], F32, kind="ExternalInput")
    d_out = nc.dram_tensor("out", [N, E], BF16, kind="ExternalOutput")

    with tile.TileContext(nc) as tc:
        _body(nc, tc, d_qT, d_kT, d_vT, d_wq, d_wk, d_wv, d_wo,
              d_bq, d_bk, d_out)

    nc.compile()
    return nc


def _body(nc, tc, d_qT, d_kT, d_vT, d_wq, d_wk, d_wv, d_wo,
          d_bq, d_bk, d_out):
    exp_f = mybir.ActivationFunctionType.Exp
    mult = mybir.AluOpType.mult

    with (
        tc.tile_pool(name="weights", bufs=1) as wpool,
        tc.tile_pool(name="acts", bufs=1) as apool,
        tc.tile_pool(name="stream", bufs=4) as spool,
        tc.tile_pool(name="exps", bufs=2) as epool,
        tc.tile_pool(name="small", bufs=3) as mpool,
    ):
        # ---- resident weights (already bf16 from host) ----
        wq_sb = wpool.tile([P, ECH, FL], BF16, tag="wq_sb")
        wk_sb = wpool.tile([P, ECH, FL], BF16, tag="wk_sb")
        wv_sb = wpool.tile([P, ECH, FL], BF16, tag="wv_sb")
        wo_sb = wpool.tile([P, 2, E], BF16, tag="wo_sb")
        bq_sb = wpool.tile([P, 2], F32, tag="bq_sb")
        bk_sb = wpool.tile([P, 2], F32, tag="bk_sb")
        # k weights first on the SP queue (first matmul needs them), q/v
        # weights on the other queues so they don't delay the k stream.
        nc.sync.dma_start(wk_sb[:], d_wk.ap().rearrange("(c p) f -> p c f", p=P))
        nc.sync.dma_start(bk_sb[:], d_bk.ap())
        nc.scalar.dma_start(wq_sb[:], d_wq.ap().rearrange("(c p) f -> p c f", p=P))
        nc.scalar.dma_start(bq_sb[:], d_bq.ap())
        nc.gpsimd.dma_start(wv_sb[:], d_wv.ap().rearrange("(c p) f -> p c f", p=P))
        nc.gpsimd.dma_start(wo_sb[:], d_wo.ap().rearrange("(t p) e -> p t e", p=P))

        # ---- persistent activations ----
        qp_sb = apool.tile([P, 2, N], BF16, tag="qp_sb")
        kp_sb = apool.tile([P, 2, N], BF16, tag="kp_sb")
        vp_sb = apool.tile([P, NT, HLOC * (D + 1)], BF16, tag="vp_sb")
        att_sb = apool.tile([P, 2, N], BF16, tag="att_sb")

        # ---- ACT table preload: tiny exp while DMAs stream ----
        aw_in = mpool.tile([P, 8], F32, tag="aw_in", name="aw_in")
        aw_out = mpool.tile([P, 8], F32, tag="aw_out", name="aw_out")
        nc.vector.memset(aw_in[:], 0.0)
        nc.scalar.activation(aw_out[:], aw_in[:], exp_f, scale=1.0)

        # ---- projections: k, v, q(block 0) ----
        with tc.tile_pool(name="ps_proj", bufs=4, space="PSUM") as pproj, \
             tc.tile_pool(name="scratch", bufs=1, space="DRAM") as dpool:
            # PE warm-up: dummy matmuls open the HAM clock gate during the
            # initial DMA stall; result DMA'd to DRAM so nothing elides it.
            warm_sb = wpool.tile([P, NB], BF16, tag="warm_sb")
            nc.vector.memset(warm_sb[:], 0.0)
            warm_ps = pproj.tile([P, NB], F32, tag="pq", name="warm_ps")
            for i in range(12):
                nc.tensor.matmul(warm_ps[:], warm_sb[:, 0:P], warm_sb[:],
                                 start=(i == 0), stop=(i == 11))
            wdump = mpool.tile([1, NB], F32, tag="wdump", name="wdump")
            nc.vector.tensor_copy(wdump[:], warm_ps[0:1, :])
            wdram = dpool.tile([1, NB], F32, tag="wdram", name="wdram")
            nc.sync.dma_start(wdram[:], wdump[:])

            def stream_in(dst, src, nb):
                for ec in range(ECH):
                    eng = (nc.sync, nc.scalar, nc.gpsimd)[ec % 3]
                    eng.dma_start(
                        dst[:, ec, :],
                        src.ap()[ec * P:(ec + 1) * P, nb * NB:(nb + 1) * NB])

            # k projection (4 blocks)
            with nc.named_scope("proj_k"):
                for nb in range(NBLK):
                    xt = spool.tile([P, ECH, NB], BF16, tag="xt", name="xt")
                    stream_in(xt, d_kT, nb)
                    pst = [pproj.tile([P, NB], F32, tag="pq",
                                      name=f"pk{ft}") for ft in range(2)]
                    for ec in range(ECH):
                        for ft in range(2):
                            nc.tensor.matmul(
                                pst[ft][:],
                                wk_sb[:, ec, ft * P:(ft + 1) * P],
                                xt[:, ec, :],
                                start=(ec == 0), stop=(ec == ECH - 1))
                    for ft in range(2):
                        nc.vector.tensor_scalar_add(
                            kp_sb[:, ft, nb * NB:(nb + 1) * NB],
                            pst[ft][:], bk_sb[:, ft:ft + 1])
            # v projection (4 blocks)
            with nc.named_scope("proj_v"):
                for nb in range(NBLK):
                    vt3 = spool.tile([P, ECH, NB], BF16, tag="xt", name="vt3")
                    stream_in(vt3, d_vT, nb)
                    for sub in range(NB // P):
                        nt_i = nb * (NB // P) + sub
                        psv = pproj.tile([P, FL], F32, tag="pq",
                                         padded_shape=[P, NB], name="psv")
                        for ec in range(ECH):
                            nc.tensor.matmul(
                                psv[:],
                                vt3[:, ec, sub * P:(sub + 1) * P],
                                wv_sb[:, ec, :],
                                start=(ec == 0), stop=(ec == ECH - 1))
                        vslc = vp_sb[:, nt_i]
                        nc.vector.tensor_copy(
                            vslc.rearrange(
                                "p (h x) -> p h x", h=HLOC)[:, :, 0:D],
                            psv[:].rearrange("p (h x) -> p h x", h=HLOC))
                        nc.vector.memset(
                            vslc.rearrange(
                                "p (h x) -> p h x", h=HLOC)[:, :, D:D + 1],
                            1.0)
            # q projection, block 0 only; blocks 1-3 prefetched into SBUF
            # now (DMA engines go idle in attention) but projected later.
            with nc.named_scope("proj_q0"):
                xq0 = spool.tile([P, ECH, NB], BF16, tag="xt", name="xq0")
                stream_in(xq0, d_qT, 0)
                qx = [spool.tile([P, ECH, NB], BF16, tag="qx", bufs=3,
                                 name=f"qx{nb}") for nb in (1, 2, 3)]
                for i, nb in enumerate((1, 2, 3)):
                    stream_in(qx[i], d_qT, nb)
                pst = [pproj.tile([P, NB], F32, tag="pq",
                                  name=f"pq{ft}") for ft in range(2)]
                for ec in range(ECH):
                    for ft in range(2):
                        nc.tensor.matmul(
                            pst[ft][:],
                            wq_sb[:, ec, ft * P:(ft + 1) * P],
                            xq0[:, ec, :],
                            start=(ec == 0), stop=(ec == ECH - 1))
                for ft in range(2):
                    nc.vector.tensor_scalar_add(
                        qp_sb[:, ft, 0:NB], pst[ft][:], bq_sb[:, ft:ft + 1])

        # ---- attention: 8 pair-iterations, ScalarE-bound ----
        with tc.tile_pool(name="ps_attn", bufs=1, space="PSUM") as pattn:
            state = {}
            fillers = deque()   # (pe_cycles, closure)
            oproj_q = deque()

            def drain(budget):
                while fillers and budget > 0:
                    c, fn = fillers.popleft()
                    fn()
                    budget -= c

            def sc_group(idx, gi):
                ib, f = ITERS[idx]
                tag, jt0, njt = SC_GROUPS[gi]
                pss = pattn.tile([P, njt, 2, NB], F32, tag=tag, name="pss")
                for u in range(njt):
                    jt = jt0 + u
                    for lo in range(2):
                        kh = kp_sb[lo * D:(lo + 1) * D, f,
                                   jt * P:(jt + 1) * P]
                        qh = qp_sb[lo * D:(lo + 1) * D, f,
                                   ib * NB:(ib + 1) * NB]
                        nc.tensor.matmul(pss[:, u, lo, :], kh, qh,
                                         start=True, stop=True)
                nc.scalar.activation(
                    state[idx][:, :, jt0:jt0 + njt, :]
                        .rearrange("p h j i -> p j h i"),
                    pss[:], exp_f, scale=SCALE)

            def av_round(idx, lo):
                # full 16-jt AV accumulation for head lo of iter idx
                # (exps[idx] is complete — runs as filler in iter idx+1)
                ib, f = ITERS[idx]
                h = 2 * f + lo
                pu = pattn.tile([D + 1, NB], F32, tag="pu",
                                padded_shape=[P, NB], name="pu")
                ex = state[idx]

                def chunk(jt0):
                    def fn():
                        for jt in range(jt0, jt0 + 4):
                            nc.tensor.matmul(
                                pu[:],
                                vp_sb[:, jt, h * (D + 1):(h + 1) * (D + 1)],
                                ex[:, lo, jt, :],
                                start=(jt == 0), stop=(jt == JT - 1))
                    return fn
                for jt0 in range(0, JT, 4):
                    fillers.append((2048, chunk(jt0)))
                fillers.append((400, lambda: norm(idx, lo, pu)))

            def norm(idx, lo, pu):
                ib, f = ITERS[idx]
                pofs = lo * D
                u_sb = mpool.tile([D + 1, NB], F32, tag="u_sb", name="u_sb")
                nc.vector.tensor_copy(u_sb[:], pu[:])
                srow = mpool.tile([1, NB], F32, tag="srow", name="srow")
                nc.vector.tensor_copy(srow[:], u_sb[D:D + 1, :])
                rec = mpool.tile([1, NB], F32, tag="rec", name="rec")
                nc.vector.reciprocal_approx_fast(rec[:], srow[:])
                rb = mpool.tile([D, NB], F32, tag="rb", name="rb")
                nc.gpsimd.partition_broadcast(rb[:], rec[:])
                nc.vector.tensor_tensor(
                    att_sb[pofs:pofs + D, f, ib * NB:(ib + 1) * NB],
                    u_sb[0:D, :], rb[:], op=mult)
                if lo == 1 and f == 1:
                    oproj_q.extend((ib * (NB // P) + s, eb)
                                   for s in range(NB // P) for eb in range(2))

            def oproj_half(tag="po"):
                if not oproj_q:
                    return
                it, eb = oproj_q.popleft()
                po = pattn.tile([P, NB], F32, tag=tag, name="po")
                for ft2 in range(2):
                    nc.tensor.matmul(
                        po[:],
                        att_sb[:, ft2, it * P:(it + 1) * P],
                        wo_sb[:, ft2, eb * NB:(eb + 1) * NB],
                        start=(ft2 == 0), stop=(ft2 == 1))
                ot = mpool.tile([P, NB], BF16, tag="ot", name="ot")
                nc.vector.tensor_copy(ot[:], po[:])
                nc.sync.dma_start(
                    d_out.ap()[it * P:(it + 1) * P,
                               eb * NB:(eb + 1) * NB],
                    ot[:])

            def qproj_chunks(nb, ft):
                # late q projection in the (still idle) out-proj PSUM
                # bank, split into two filler-sized accumulation chunks
                hold = {}

                def c1():
                    pq = pattn.tile([P, NB], F32, tag="po", name="pql")
                    hold["pq"] = pq
                    for ec in range(4):
                        nc.tensor.matmul(
                            pq[:], wq_sb[:, ec, ft * P:(ft + 1) * P],
                            qx[nb - 1][:, ec, :],
                            start=(ec == 0), stop=False)

                def c2():
                    pq = hold["pq"]
                    for ec in range(4, ECH):
                        nc.tensor.matmul(
                            pq[:], wq_sb[:, ec, ft * P:(ft + 1) * P],
                            qx[nb - 1][:, ec, :],
                            start=False, stop=(ec == ECH - 1))
                    nc.vector.tensor_scalar_add(
                        qp_sb[:, ft, nb * NB:(nb + 1) * NB],
                        pq[:], bq_sb[:, ft:ft + 1])
                return [(2200, c1), (2400, c2)]

            qchunks = deque()
            for nb in (1, 2, 3):
                for ft in range(2):
                    qchunks.extend(qproj_chunks(nb, ft))

            for idx in range(len(ITERS)):
                with nc.named_scope(f"attn_i{idx}"):
                    # leftovers from 2 iters back must land before g0's
                    # ACT reuses their exps slot
                    drain(1 << 30)
                    state[idx] = epool.tile([P, 2, JT, NB], BF16, tag="exps",
                                            name="exps")
                    if idx >= 1:
                        av_round(idx - 1, 0)
                        av_round(idx - 1, 1)
                    if idx == 0:
                        for _ in range(8):
                            if qchunks:
                                fillers.append(qchunks.popleft())
                    elif idx == 1:
                        while qchunks:
                            fillers.append(qchunks.popleft())
                    for gi, (tag, jt0, njt) in enumerate(SC_GROUPS):
                        sc_group(idx, gi)
                        drain(2300 if tag == "psA" else 1500)
                        if gi in (3, 5, 7, 9):
                            oproj_half()
                            oproj_half()
                    if idx >= 2:
                        del state[idx - 2]

            # ---- tail: last iter's AV/norm + final out-proj drain ----
            idx = len(ITERS) - 1
            drain(1 << 30)
            av_round(idx, 0)
            av_round(idx, 1)
            drain(1 << 30)
            tags = ["po", "psB", "psA"]
            t = 0
            while oproj_q:
                oproj_half(tags[t % 3])
                t += 1


_CACHE = {}


def _shard_inputs(q, k, v, Wq, bq, Wk, bk, Wv, Wo):
    import ml_dtypes
    bf = ml_dtypes.bfloat16
    in_maps = []
    for c in range(8):
        b, g = divmod(c, 4)
        fs = slice(g * FL, (g + 1) * FL)
        in_maps.append({
            "qT": np.ascontiguousarray(q[b].T.astype(bf)),
            "kT": np.ascontiguousarray(k[b].T.astype(bf)),
            "vT": np.ascontiguousarray(v[b].T.astype(bf)),
            "wq": np.ascontiguousarray(Wq[fs, :].T.astype(bf)),
            "wk": np.ascontiguousarray(Wk[fs, :].T.astype(bf)),
            "wv": np.ascontiguousarray(Wv[fs, :].T.astype(bf)),
            "wo": np.ascontiguousarray(Wo[:, fs].T.astype(bf)),
            "bq": np.ascontiguousarray(bq[fs].reshape(2, P).T.astype(F32_NP)),
            "bk": np.ascontiguousarray(bk[fs].reshape(2, P).T.astype(F32_NP)),
        })
    return in_maps


def kernel(q, k, v, Wq, bq, Wk, bk, Wv, bv, Wo, bo):
    from concourse import bass_utils

    q = np.asarray(q, F32_NP)
    k = np.asarray(k, F32_NP)
    v = np.asarray(v, F32_NP)
    Wq = np.asarray(Wq, F32_NP)
    Wk = np.asarray(Wk, F32_NP)
    Wv = np.asarray(Wv, F32_NP)
    Wo = np.asarray(Wo, F32_NP)
    bq = np.asarray(bq, F32_NP)
    bk = np.asarray(bk, F32_NP)
    bv = np.asarray(bv, F32_NP)
    bo = np.asarray(bo, F32_NP)

    if "nc" not in _CACHE:
        _CACHE["nc"] = build()
    nc = _CACHE["nc"]

    in_maps = _shard_inputs(q, k, v, Wq, bq, Wk, bk, Wv, Wo)
    res = bass_utils.run_bass_kernel_spmd(nc, in_maps, core_ids=list(range(8)))

    extra = (Wo @ bv + bo).astype(F32_NP)
    out = np.zeros((B, N, E), F32_NP)
    for b in range(B):
        acc = np.zeros((N, E), F32_NP)
        for g in range(4):
            acc += res.results[b * 4 + g]["out"].astype(F32_NP)
        out[b] = acc + extra
    return out
